# revision 1
# baseline (speedup 1.0000x reference)
"""GQA attention kernel for Trainium2 (Bass/Tile), 8-core SPMD.

Problem: B=2, N=2048, DIM=1024, 16 query heads / 4 KV heads, head_dim=64, fp32.
Sharding: core c = (batch b=c//4, kv-group g=c%4). Each core computes its
group's 4 query heads + 1 shared KV head over the full sequence, and a partial
output projection (its 256 rows of Wo). Host sums the 4 group partials per
batch and adds the bias.

Per-core layout (all "T" tensors keep head_dim/feature on partitions, seq on
free dim):
  xT   [128, N] x 8     : x^T, from PE transposes of DMA'd x tiles
  qt_p [128, N] x 2     : Q^T head pairs (head 2p on partitions 0-63, 2p+1 on 64-127)
  kkT  [128, N]         : K^T duplicated (rows 0-63 == 64-127) to feed row-paired
                          score matmuls for both heads of a pair
  vn   [128, 16, 64] bf16 : V in normal layout (seq on partitions), for P@V
Scores are computed transposed (S^T tile [128 keys, 512 queries]) so softmax
needs no max-subtraction (scores bounded ~|8|) and exp output P^T feeds P@V
directly.  Sum-of-exp per query rides on 4-way column-tiled ones-matmuls.
"""

import sys

if "/opt/trn_rl_repo" not in sys.path:
    sys.path.insert(0, "/opt/trn_rl_repo")

from contextlib import ExitStack

import numpy as np

import concourse.bass as bass
import concourse.mybir as mybir
import concourse.tile as tile
from concourse import bacc, bass_utils
from concourse.bass import ds, ts
from concourse.masks import make_identity

F32 = mybir.dt.float32
F32R = mybir.dt.float32r
BF16 = mybir.dt.bfloat16
EXPF = mybir.ActivationFunctionType.Exp

DIM = 1024
D = 64  # head dim
SCALE = D ** -0.5


def build_nc(NSEQ=2048):
    KT = NSEQ // 128   # key tiles
    QC = NSEQ // 512   # query chunks of 512
    DKT = DIM // 128   # contraction tiles for projections

    nc = bacc.Bacc("TRN2", target_bir_lowering=False, debug=False)
    x = nc.dram_tensor("x", [NSEQ, DIM], F32, kind="ExternalInput").ap()
    wq = nc.dram_tensor("wq", [DIM, 256], F32, kind="ExternalInput").ap()
    wk = nc.dram_tensor("wk", [DIM, D], F32, kind="ExternalInput").ap()
    wv = nc.dram_tensor("wv", [DIM, D], F32, kind="ExternalInput").ap()
    wo = nc.dram_tensor("wo", [256, DIM], F32, kind="ExternalInput").ap()
    out = nc.dram_tensor("out", [DIM, NSEQ], F32, kind="ExternalOutput").ap()
    scr = nc.dram_tensor("scr", [QC, 4, 512], F32, kind="Internal").ap()

    with tile.TileContext(nc) as tc, ExitStack() as ctx:
        sb = ctx.enter_context(tc.tile_pool(name="sb", bufs=1))

        wq_sb = sb.tile([128, DKT, 256], F32R)
        wkk_sb = sb.tile([128, DKT, 128], F32R)
        wv_sb = sb.tile([128, DKT, D], F32R)
        wo_sb = sb.tile([128, 2, DIM], F32R)
        ident = sb.tile([128, 128], F32)
        ones_k = sb.tile([128, 1], BF16)
        warm = sb.tile([128, 1], F32)

        nc.sync.dma_start(out=wq_sb, in_=wq.rearrange("(t p) m -> p t m", p=128).bitcast(F32R))
        nc.sync.dma_start(out=wkk_sb[:, :, 0:D], in_=wk.rearrange("(t p) m -> p t m", p=128).bitcast(F32R))
        nc.sync.dma_start(out=wkk_sb[:, :, D:128], in_=wk.rearrange("(t p) m -> p t m", p=128).bitcast(F32R))
        nc.sync.dma_start(out=wv_sb, in_=wv.rearrange("(t p) m -> p t m", p=128).bitcast(F32R))
        nc.sync.dma_start(out=wo_sb, in_=wo.rearrange("(t p) m -> p t m", p=128).bitcast(F32R))
        make_identity(nc, ident)
        nc.vector.memset(ones_k, 1.0)
        # preload the exp table set off the critical path
        nc.scalar.activation(out=warm, in_=ones_k, func=EXPF, scale=1.0)

        xT = [sb.tile([128, NSEQ], F32R, name=f"xT{d}") for d in range(DKT)]
        qt = [sb.tile([128, NSEQ], F32R, name=f"qt{p}") for p in range(2)]
        kkT = sb.tile([128, NSEQ], F32R)
        vT = sb.tile([64, NSEQ], F32)
        vn1 = sb.tile([128, KT, D + 1], BF16)
        nc.vector.memset(vn1, 1.0)
        aout = [sb.tile([128, NSEQ], F32R, name=f"aout{p}") for p in range(2)]

        # stage pools: xpool only (psum unified with attention pools below)
        xpool = ctx.enter_context(tc.tile_pool(name="xp", bufs=5))

        # ---------------- stage 2: attention loop ----------------
        ps_s = ctx.enter_context(tc.tile_pool(name="ps_s", bufs=2, space="PSUM"))
        ps_pv = ctx.enter_context(tc.tile_pool(name="ps_pv", bufs=4, space="PSUM"))

        def emit_sgroup(sg):
            """Load+transpose x chunk sg; project K/V for that key chunk."""
            xs = [xpool.tile([128, DIM], F32, tag="xs", name=f"xs{sg}_{_i}") for _i in range(4)]
            for i in range(4):
                nc.sync.dma_start(out=xs[i], in_=x[ts(sg * 4 + i, 128), :])
            for d in range(DKT):
                ptr = ps_s.tile([128, 1024], F32, tag="sc", name=f"ptr{sg}_{d}")
                for i in range(4):
                    nc.tensor.transpose(ptr[:, ts(i, 128)], xs[i][:, ts(d, 128)], ident)
                nc.vector.tensor_copy(xT[d][:, ds(sg * 512, 512)], ptr[:, 0:512])
            pk = ps_s.tile([128, 1024], F32, tag="sc", name=f"pk{sg}")
            for d in range(DKT):
                nc.tensor.matmul(pk[:, 0:512], wkk_sb[:, d, :], xT[d][:, ds(sg * 512, 512)],
                                 start=(d == 0), stop=(d == DKT - 1))
            nc.vector.tensor_copy(kkT[:, ds(sg * 512, 512)], pk[:, 0:512])
            pv_ = ps_s.tile([128, 1024], F32, tag="sc", name=f"pvp{sg}")
            for d in range(DKT):
                nc.tensor.matmul(pv_[0:64, 0:512], wv_sb[:, d, :], xT[d][:, ds(sg * 512, 512)],
                                 start=(d == 0), stop=(d == DKT - 1))
            nc.vector.tensor_copy(vT[:, ds(sg * 512, 512)], pv_[0:64, 0:512])
            ptv = ps_s.tile([128, 1024], F32, tag="sc", name=f"ptv{sg}")
            for i in range(4):
                t = sg * 4 + i
                nc.tensor.transpose(ptv[:, ds(i * D, D)], vT[:, ts(t, 128)], ident[0:64, 0:64])
            nc.vector.tensor_copy(vn1[:, sg * 4:(sg + 1) * 4, 0:D], ptv[:, 0:4 * D])

        def emit_qt(qc):
            for p in range(2):
                pq = ps_s.tile([128, 1024], F32, tag="sc", name=f"pq{qc}_{p}")
                for d in range(DKT):
                    nc.tensor.matmul(pq[:, 0:512], wq_sb[:, d, ts(p, 128)], xT[d][:, ds(qc * 512, 512)],
                                     start=(d == 0), stop=(d == DKT - 1))
                nc.vector.tensor_copy(qt[p][:, ds(qc * 512, 512)], pq[:, 0:512])
        ptp = ctx.enter_context(tc.tile_pool(name="ptp", bufs=10))
        rrp = ctx.enter_context(tc.tile_pool(name="rrp", bufs=2))
        Rp_pool = ctx.enter_context(tc.tile_pool(name="Rp", bufs=4))
        outp = ctx.enter_context(tc.tile_pool(name="outp", bufs=3))

        state = {}

        def emit_norm(qc):
            pvs = state[qc]
            rr = rrp.tile([128, 2048], F32, tag="rr")
            for h in range(4):
                nc.vector.reciprocal(out=rr[ds(64, 1), ds(h * 512, 512)], in_=pvs[h][ds(64, 1), :])
            r64 = rr[ds(64, 1), :]
            nc.sync.dma_start(
                out=scr[qc:qc + 1, :, :],
                in_=bass.AP(tensor=r64.tensor, offset=r64.offset,
                            ap=[[r64.ap[0][0], 1], [512, 4], [1, 512]]),
            )
            for p in range(2):
                Rt = Rp_pool.tile([128, 512], F32, tag="R")
                for i in range(2):
                    src = bass.AP(tensor=scr.tensor,
                                  offset=scr.offset + (qc * 4 + 2 * p + i) * 512,
                                  ap=[[0, 64], [1, 512]])
                    nc.sync.dma_start(out=Rt[ds(i * 64, 64), :], in_=src)
                for i in range(2):
                    nc.vector.tensor_mul(aout[p][ds(i * 64, 64), ds(qc * 512, 512)],
                                         pvs[2 * p + i][0:64, :], Rt[ds(i * 64, 64), :])

        def emit_outproj(qc):
            for od in range(DIM // 128):
                po = ps_s.tile([128, 512], F32, tag="sc")
                nc.tensor.matmul(po, wo_sb[:, 0, ts(od, 128)], aout[0][:, ds(qc * 512, 512)],
                                 start=True, stop=False)
                nc.tensor.matmul(po, wo_sb[:, 1, ts(od, 128)], aout[1][:, ds(qc * 512, 512)],
                                 start=False, stop=True)
                ot = outp.tile([128, 512], F32, tag="ot")
                nc.vector.tensor_copy(ot, po)
                nc.sync.dma_start(out=out[ts(od, 128), ds(qc * 512, 512)], in_=ot)

        pending_pv = []

        def flush_pv():
            for (qc_, j_, h_, pt_) in pending_pv:
                for t in range(2):
                    kt = 2 * j_ + t
                    nc.tensor.matmul(state[qc_][h_][0:65, :],
                                     vn1[:, kt, :], pt_[:, ds(t * 512, 512)],
                                     start=(kt == 0), stop=(kt == KT - 1))
            pending_pv.clear()

        def emit_quanta(qc, j):
            new_pv = []
            for h in range(4):
                p, i = h // 2, h % 2
                psc = ps_s.tile([128, 1024], F32, tag="sc", name=f"psc{qc}_{j}_{h}")
                for t in range(2):
                    kt = 2 * j + t
                    nc.tensor.matmul(psc[:, ds(t * 512, 512)],
                                     kkT[ds(i * 64, 64), ts(kt, 128)],
                                     qt[p][ds(i * 64, 64), ds(qc * 512, 512)],
                                     start=True, stop=True)
                pt = ptp.tile([128, 1024], BF16, tag="pt", name=f"pt{qc}_{j}_{h}")
                nc.scalar.activation(out=pt, in_=psc, func=EXPF, scale=SCALE)
                new_pv.append((qc, j, h, pt))
            flush_pv()
            pending_pv.extend(new_pv)

        # interleaved prologue: per key chunk, project K/V then run qc=0 attention on it
        state[0] = [ps_pv.tile([128, 512], F32, tag="pv", name=f"pv0_{h}") for h in range(4)]
        for sg in range(QC):
            emit_sgroup(sg)
            if sg == 0:
                emit_qt(0)
            emit_quanta(0, 2 * sg)
            emit_quanta(0, 2 * sg + 1)
        flush_pv()
        emit_norm(0)
        for qc in range(1, QC):
            pvs = [ps_pv.tile([128, 512], F32, tag="pv", name=f"pv{qc}_{h}") for h in range(4)]
            state[qc] = pvs
            emit_qt(qc)
            for j in range(KT // 2):
                emit_quanta(qc, j)
                if j == 1:
                    emit_outproj(qc - 1)
            flush_pv()
            emit_norm(qc)
        emit_outproj(QC - 1)

    nc.compile()
    return nc


_CACHE = {}


def _get_nc(NSEQ):
    if NSEQ not in _CACHE:
        _CACHE[NSEQ] = build_nc(NSEQ)
    return _CACHE[NSEQ]


def kernel(x, Wq, Wk, Wv, Wo, bo):
    """Full-input entry point: shard over 8 cores, run, gather."""
    x, Wq, Wk, Wv, Wo, bo = (np.asarray(a, np.float32) for a in (x, Wq, Wk, Wv, Wo, bo))
    B, N, C = x.shape
    nc = _get_nc(N)
    in_maps = []
    for c in range(8):
        b, g = c // 4, c % 4
        in_maps.append({
            "x": np.ascontiguousarray(x[b]),
            "wq": np.ascontiguousarray(Wq[:, g * 256:(g + 1) * 256]),
            "wk": np.ascontiguousarray(Wk[:, g * D:(g + 1) * D]),
            "wv": np.ascontiguousarray(Wv[:, g * D:(g + 1) * D]),
            "wo": np.ascontiguousarray(Wo[g * 256:(g + 1) * 256, :]),
        })
    res = bass_utils.run_bass_kernel_spmd(nc, in_maps, core_ids=list(range(8)))
    outs = [res.results[c]["out"] for c in range(8)]
    full = np.empty((B, N, C), np.float32)
    for b in range(B):
        acc = outs[4 * b].astype(np.float32)
        for g in range(1, 4):
            acc = acc + outs[4 * b + g]
        full[b] = acc.T + bo[None, :]
    return full



# revision 7
# speedup vs baseline: 1.0516x; 1.0516x over previous
"""GQA attention kernel for Trainium2 (Bass/Tile), 8-core SPMD.

Problem: B=2, N=2048, DIM=1024, 16 query heads / 4 KV heads, head_dim=64, fp32.
Sharding: core c = (batch b=c//4, kv-group g=c%4). Each core computes its
group's 4 query heads + 1 shared KV head over the full sequence, and a partial
output projection (its 256 rows of Wo). Host sums the 4 group partials per
batch and adds the bias.

Layout per core:
  xT    [128, 8, N] f32r : x^T (PE transposes with an f32r identity: 1.5 cyc/row)
  qt    [128, 2, N] f32r : Q^T head pairs (head 2p on partitions 0-63, 2p+1 on
                           64-127)
  kkT   [128, N]    f32r : K^T duplicated across partition halves (DMA dup)
  vn    [128, 16, 65] bf16: V in normal layout (keys on partitions) + ones col
  aoutT [128, 2, N] f32r : normalized attention out^T for the out-projection

Scores are computed transposed (S^T [128 keys, 512 queries]); exp on Act; P@V
uses P^T tiles as the *stationary* operand and [V | 1] as the moving operand,
producing [queries, 65] in PSUM (col 64 = sum-of-exp) at 65 rows/matmul
instead of 128. K and V projections are fused into one 128-column matmul.
"""

import sys

if "/opt/trn_rl_repo" not in sys.path:
    sys.path.insert(0, "/opt/trn_rl_repo")

from contextlib import ExitStack

import numpy as np

import concourse.bass as bass
import concourse.mybir as mybir
import concourse.tile as tile
from concourse import bacc, bass_utils
from concourse.bass import ds, ts
from concourse.masks import make_identity

F32 = mybir.dt.float32
F32R = mybir.dt.float32r
BF16 = mybir.dt.bfloat16
EXPF = mybir.ActivationFunctionType.Exp

DIM = 1024
D = 64  # head dim
SCALE = D ** -0.5


def build_nc(NSEQ=2048):
    KT = NSEQ // 128   # key tiles
    QC = NSEQ // 512   # query chunks of 512
    DKT = DIM // 128   # contraction tiles for projections

    nc = bacc.Bacc("TRN2", target_bir_lowering=False, debug=False)
    x = nc.dram_tensor("x", [NSEQ, DIM], F32, kind="ExternalInput").ap()
    wq = nc.dram_tensor("wq", [DIM, 256], F32, kind="ExternalInput").ap()
    wkv = nc.dram_tensor("wkv", [DIM, 128], F32, kind="ExternalInput").ap()
    wo = nc.dram_tensor("wo", [256, DIM], F32, kind="ExternalInput").ap()
    out = nc.dram_tensor("out", [DIM, NSEQ], F32, kind="ExternalOutput").ap()

    xr = x.bitcast(F32R)

    with tile.TileContext(nc) as tc, ExitStack() as ctx:
        sb = ctx.enter_context(tc.tile_pool(name="sb", bufs=1))

        wq_sb = sb.tile([128, DKT, 256], F32R)
        wkv_sb = sb.tile([128, DKT, 128], F32R)
        wo_sb = sb.tile([128, 2, DIM], F32R)
        ident = sb.tile([128, 128], F32)
        identr = sb.tile([128, 128], F32R)
        warm_in = sb.tile([128, 1], F32)
        warm = sb.tile([128, 1], F32)

        nc.sync.dma_start(out=wq_sb, in_=wq.rearrange("(t p) m -> p t m", p=128).bitcast(F32R))
        nc.sync.dma_start(out=wkv_sb, in_=wkv.rearrange("(t p) m -> p t m", p=128).bitcast(F32R))
        nc.sync.dma_start(out=wo_sb, in_=wo.rearrange("(t p) m -> p t m", p=128).bitcast(F32R))
        make_identity(nc, ident)
        nc.vector.tensor_copy(identr, ident)
        nc.vector.memset(warm_in, 1.0)
        # preload the exp table set off the critical path
        nc.scalar.activation(out=warm, in_=warm_in, func=EXPF, scale=1.0)

        xT = sb.tile([128, DKT, NSEQ], F32R)
        qt = sb.tile([128, 2, NSEQ], F32R)
        kkT = sb.tile([128, NSEQ], F32R)
        vn = sb.tile([128, KT, D + 1], BF16)
        aoutT = sb.tile([128, 2, NSEQ], F32R)
        nc.vector.memset(vn, 1.0)

        xpool = ctx.enter_context(tc.tile_pool(name="xp", bufs=5))
        vtp = ctx.enter_context(tc.tile_pool(name="vtp", bufs=2))
        ptp = ctx.enter_context(tc.tile_pool(name="ptp", bufs=10))
        rrp = ctx.enter_context(tc.tile_pool(name="rrp", bufs=2))
        aop = ctx.enter_context(tc.tile_pool(name="aop", bufs=2))
        outp = ctx.enter_context(tc.tile_pool(name="outp", bufs=3))
        ps_s = ctx.enter_context(tc.tile_pool(name="ps_s", bufs=2, space="PSUM"))
        ps_pv = ctx.enter_context(tc.tile_pool(name="ps_pv", bufs=4, space="PSUM"))

        state = {}
        pending_pv = []

        def emit_sgroup(sg):
            """Load+transpose x chunk sg; project K|V fused for that key chunk."""
            xs = [xpool.tile([128, DIM], F32R, tag="xs", name=f"xs{sg}_{i}") for i in range(4)]
            for i in range(4):
                nc.sync.dma_start(out=xs[i], in_=xr[ts(sg * 4 + i, 128), :])
            for d in range(DKT):
                ptr = ps_s.tile([128, 1024], F32R, tag="sc", name=f"ptr{sg}_{d}")
                for i in range(4):
                    nc.tensor.transpose(ptr[:, ts(i, 128)], xs[i][:, ts(d, 128)], identr)
                nc.vector.tensor_copy(xT[:, d, ds(sg * 512, 512)], ptr[:, 0:512])
            pkv = ps_s.tile([128, 1024], F32, tag="sc", name=f"pkv{sg}")
            for d in range(DKT):
                nc.tensor.matmul(pkv[:, 0:512], wkv_sb[:, d, :], xT[:, d, ds(sg * 512, 512)],
                                 start=(d == 0), stop=(d == DKT - 1))
            nc.vector.tensor_copy(kkT[ds(0, 64), ds(sg * 512, 512)], pkv[ds(0, 64), 0:512])
            nc.sync.dma_start(out=kkT[ds(64, 64), ds(sg * 512, 512)],
                              in_=kkT[ds(0, 64), ds(sg * 512, 512)])
            vtmp = vtp.tile([64, 512], F32R, tag="vt", name=f"vt{sg}")
            nc.vector.tensor_copy(vtmp, pkv[ds(64, 64), 0:512])
            ptv = ps_s.tile([128, 1024], F32R, tag="sc", name=f"ptv{sg}")
            for i in range(4):
                nc.tensor.transpose(ptv[:, ds(i * D, D)], vtmp[:, ts(i, 128)], identr[0:64, 0:64])
            nc.vector.tensor_copy(vn[:, ds(sg * 4, 4), 0:D], ptv[:, 0:4 * D])

        def emit_qt(qc, ps=(0, 1)):
            for p in ps:
                pq = ps_s.tile([128, 1024], F32, tag="sc", name=f"pq{qc}_{p}")
                for d in range(DKT):
                    nc.tensor.matmul(pq[:, 0:512], wq_sb[:, d, ts(p, 128)],
                                     xT[:, d, ds(qc * 512, 512)],
                                     start=(d == 0), stop=(d == DKT - 1))
                nc.vector.tensor_copy(qt[:, p, ds(qc * 512, 512)], pq[:, 0:512])

        def flush_pv():
            for (qc_, j_, h_, pt_) in pending_pv:
                for t in range(2):
                    kt = 2 * j_ + t
                    for i in range(4):
                        # start=True zeroes the whole 2KB PSUM bank, so only
                        # the first sub-tile of kt==0 may set it; the other
                        # sub-series accumulate onto the zeroed bank.
                        nc.tensor.matmul(state[qc_][h_][:, ds(i * 65, 65)],
                                         pt_[:, ds(t * 512 + i * 128, 128)],
                                         vn[:, kt, :],
                                         start=(kt == 0 and i == 0),
                                         stop=(kt == KT - 1),
                                         skip_group_check=True)
            pending_pv.clear()

        def emit_quanta(qc, j):
            new_pv = []
            for h in range(4):
                p, i = h // 2, h % 2
                psc = ps_s.tile([128, 1024], F32, tag="sc", name=f"psc{qc}_{j}_{h}")
                for t in range(2):
                    kt = 2 * j + t
                    nc.tensor.matmul(psc[:, ds(t * 512, 512)],
                                     kkT[ds(i * 64, 64), ts(kt, 128)],
                                     qt[ds(i * 64, 64), p, ds(qc * 512, 512)],
                                     start=True, stop=True)
                pt = ptp.tile([128, 1024], BF16, tag="pt", name=f"pt{qc}_{j}_{h}")
                nc.scalar.activation(out=pt, in_=psc, func=EXPF, scale=SCALE)
                new_pv.append((qc, j, h, pt))
            flush_pv()
            pending_pv.extend(new_pv)

        def emit_norm(qc):
            pvs = state[qc]
            ao = aop.tile([128, 4, 4, D], F32R, tag="ao", name=f"ao{qc}")
            rr = rrp.tile([128, 4, 4], F32, tag="rr", name=f"rr{qc}")  # [*, h, sub]
            for h in range(4):
                t = pvs[h]
                src = bass.AP(tensor=t.tensor, offset=t.offset + 64,
                              ap=[[t.ap[0][0], 128], [65, 4]])
                nc.vector.reciprocal(out=rr[:, h, :], in_=src)
            for h in range(4):
                for i in range(4):
                    nc.vector.tensor_scalar_mul(ao[:, i, h, :], pvs[h][:, ds(i * 65, 64)],
                                          rr[:, h, ds(i, 1)])
            for hp in range(2):
                pat = ps_s.tile([128, 1024], F32R, tag="sc", name=f"pat{qc}_{hp}")
                for i in range(4):
                    nc.tensor.transpose(pat[:, ds(i * 128, 128)],
                                        ao[:, i, ds(2 * hp, 2), :], identr)
                nc.vector.tensor_copy(aoutT[:, hp, ds(qc * 512, 512)], pat[:, 0:512])

        def emit_outproj(qc, ods):
            for od in ods:
                po = ps_s.tile([128, 1024], F32, tag="sc", name=f"po{qc}_{od}")
                nc.tensor.matmul(po[:, 0:512], wo_sb[:, 0, ts(od, 128)],
                                 aoutT[:, 0, ds(qc * 512, 512)], start=True, stop=False)
                nc.tensor.matmul(po[:, 0:512], wo_sb[:, 1, ts(od, 128)],
                                 aoutT[:, 1, ds(qc * 512, 512)], start=False, stop=True)
                ot = outp.tile([128, 512], F32, tag="ot", name=f"ot{qc}_{od}")
                nc.vector.tensor_copy(ot, po[:, 0:512])
                nc.sync.dma_start(out=out[ts(od, 128), ds(qc * 512, 512)], in_=ot)

        # interleaved prologue: per key chunk, project K/V then run qc=0
        # attention on it
        state[0] = [ps_pv.tile([128, 512], F32, tag="pv", name=f"pv0_{h}") for h in range(4)]
        for sg in range(QC):
            emit_sgroup(sg)
            if sg == 0:
                emit_qt(0)
            emit_quanta(0, 2 * sg)
            emit_quanta(0, 2 * sg + 1)
        flush_pv()
        for qc in range(1, QC):
            emit_qt(qc, ps=(0,))
            emit_norm(qc - 1)
            emit_qt(qc, ps=(1,))
            state[qc] = [ps_pv.tile([128, 512], F32, tag="pv", name=f"pv{qc}_{h}")
                         for h in range(4)]
            for j in range(KT // 2):
                emit_quanta(qc, j)
                if j == 1:
                    emit_outproj(qc - 1, range(0, 4))
                if j == 4:
                    emit_outproj(qc - 1, range(4, 8))
            flush_pv()
        emit_norm(QC - 1)
        emit_outproj(QC - 1, range(0, 8))

    nc.compile()
    return nc


_CACHE = {}


def _get_nc(NSEQ):
    if NSEQ not in _CACHE:
        _CACHE[NSEQ] = build_nc(NSEQ)
    return _CACHE[NSEQ]


def kernel(x, Wq, Wk, Wv, Wo, bo):
    """Full-input entry point: shard over 8 cores, run, gather."""
    x, Wq, Wk, Wv, Wo, bo = (np.asarray(a, np.float32) for a in (x, Wq, Wk, Wv, Wo, bo))
    B, N, C = x.shape
    nc = _get_nc(N)
    in_maps = []
    for c in range(8):
        b, g = c // 4, c % 4
        in_maps.append({
            "x": np.ascontiguousarray(x[b]),
            "wq": np.ascontiguousarray(Wq[:, g * 256:(g + 1) * 256]),
            "wkv": np.ascontiguousarray(np.concatenate(
                [Wk[:, g * D:(g + 1) * D], Wv[:, g * D:(g + 1) * D]], axis=1)),
            "wo": np.ascontiguousarray(Wo[g * 256:(g + 1) * 256, :]),
        })
    res = bass_utils.run_bass_kernel_spmd(nc, in_maps, core_ids=list(range(8)))
    outs = [res.results[c]["out"] for c in range(8)]
    full = np.empty((B, N, C), np.float32)
    for b in range(B):
        acc = outs[4 * b].astype(np.float32)
        for g in range(1, 4):
            acc = acc + outs[4 * b + g]
        full[b] = acc.T + bo[None, :]
    return full


# revision 9
# speedup vs baseline: 1.1486x; 1.0923x over previous
"""GQA attention kernel for Trainium2 (Bass/Tile), 8-core SPMD.

Problem: B=2, N=2048, DIM=1024, 16 query heads / 4 KV heads, head_dim=64, fp32.
Sharding: core c = (batch b=c//4, kv-group g=c%4). Each core computes its
group's 4 query heads + 1 shared KV head over the full sequence, and a partial
output projection (its 256 rows of Wo). Host sums the 4 group partials per
batch and adds the bias.

Layout per core:
  xT    [128, 8, N] f32r : x^T (PE transposes with an f32r identity)
  qt    [128, 2, N] f32r : Q^T head pairs (head 2p on partitions 0-63, 2p+1 on
                           64-127)
  kkT   [128, N]    f32r : K^T duplicated across partition halves (DMA dup)
  vn    [128, 16, 65] bf16: V in normal layout (keys on partitions) + ones col
  aoutT [128, 2, N] f32r : normalized attention out^T for the out-projection

Scores are computed transposed (S^T [128 keys, 512 queries]); exp on Act; P@V
uses P^T tiles as the *stationary* operand and V as the moving operand,
producing [queries, 64] in PSUM at 64 rows/matmul instead of 128; sum-of-exp
rides on 1-row ones-matmuls into a dedicated PSUM bank.

PSUM budget (8 banks): scores 2x[128,1024] double-buffered (4) + P@V
accumulators 2x[128,512] (2, two heads per bank) + transpose staging (1) +
sum-of-exp (1). Projection matmuls share the score pool, interleaved
fine-grained between score tiles so no engine convoys behind one pool.
"""

import sys

if "/opt/trn_rl_repo" not in sys.path:
    sys.path.insert(0, "/opt/trn_rl_repo")

from collections import deque
from contextlib import ExitStack

import numpy as np

import concourse.bass as bass
import concourse.mybir as mybir
import concourse.tile as tile
from concourse import bacc, bass_utils
from concourse.bass import ds, ts
from concourse.masks import make_identity

F32 = mybir.dt.float32
F32R = mybir.dt.float32r
BF16 = mybir.dt.bfloat16
EXPF = mybir.ActivationFunctionType.Exp

DIM = 1024
D = 64  # head dim
SCALE = D ** -0.5


def build_nc(NSEQ=2048):
    KT = NSEQ // 128   # key tiles
    QC = NSEQ // 512   # query chunks of 512
    DKT = DIM // 128   # contraction tiles for projections

    nc = bacc.Bacc("TRN2", target_bir_lowering=False, debug=False)
    x = nc.dram_tensor("x", [NSEQ, DIM], F32, kind="ExternalInput").ap()
    wq = nc.dram_tensor("wq", [DIM, 256], F32, kind="ExternalInput").ap()
    wkv = nc.dram_tensor("wkv", [DIM, 128], F32, kind="ExternalInput").ap()
    wo = nc.dram_tensor("wo", [256, DIM], F32, kind="ExternalInput").ap()
    out = nc.dram_tensor("out", [DIM, NSEQ], F32, kind="ExternalOutput").ap()

    xr = x.bitcast(F32R)

    with tile.TileContext(nc) as tc, ExitStack() as ctx:
        sb = ctx.enter_context(tc.tile_pool(name="sb", bufs=1))

        wq_sb = sb.tile([128, DKT, 256], F32R)
        wkv_sb = sb.tile([128, DKT, 128], F32R)
        wo_sb = sb.tile([128, 2, DIM], F32R)
        ident = sb.tile([128, 128], F32)
        identr = sb.tile([128, 128], F32R)
        warm_in = sb.tile([128, 1], F32)
        warm = sb.tile([128, 1], F32)

        nc.sync.dma_start(out=wq_sb, in_=wq.rearrange("(t p) m -> p t m", p=128).bitcast(F32R))
        nc.sync.dma_start(out=wkv_sb, in_=wkv.rearrange("(t p) m -> p t m", p=128).bitcast(F32R))
        nc.sync.dma_start(out=wo_sb, in_=wo.rearrange("(t p) m -> p t m", p=128).bitcast(F32R))
        make_identity(nc, ident)
        nc.vector.tensor_copy(identr, ident)
        nc.vector.memset(warm_in, 1.0)
        # preload the exp table set off the critical path
        nc.scalar.activation(out=warm, in_=warm_in, func=EXPF, scale=1.0)

        xT = sb.tile([128, DKT, NSEQ], F32R)
        qt = sb.tile([128, 2, NSEQ], F32R)
        kkT = sb.tile([128, NSEQ], F32R)
        vn = sb.tile([128, KT, D + 1], BF16)
        aoutT = sb.tile([128, 2, NSEQ], F32R)
        nc.vector.memset(vn, 1.0)

        xpool = ctx.enter_context(tc.tile_pool(name="xp", bufs=6))
        vtp = ctx.enter_context(tc.tile_pool(name="vtp", bufs=2))
        ptp = ctx.enter_context(tc.tile_pool(name="ptp", bufs=10))
        rrp = ctx.enter_context(tc.tile_pool(name="rrp", bufs=2))
        aop = ctx.enter_context(tc.tile_pool(name="aop", bufs=2))
        outp = ctx.enter_context(tc.tile_pool(name="outp", bufs=3))
        # PSUM: ps_sc 2x[128,1024] (banks 0-3), ps_pv 2x[128,512] (4-5),
        # ps_pj 1x[128,512] (6), ps_su 1x[128,16] (7)
        ps_sc = ctx.enter_context(tc.tile_pool(name="ps_sc", bufs=2, space="PSUM"))
        ps_pv = ctx.enter_context(tc.tile_pool(name="ps_pv", bufs=2, space="PSUM"))
        ps_pj = ctx.enter_context(tc.tile_pool(name="ps_pj", bufs=1, space="PSUM"))
        ps_su = ctx.enter_context(tc.tile_pool(name="ps_su", bufs=1, space="PSUM"))

        state = {}     # qc -> [hp0_tile, hp1_tile] each [128, 2, 4, 64] view
        sums = {}      # qc -> [128, 16] psum tile (cols h*4+i)
        pending_pv = []
        fillq = deque()

        def fill(n=1):
            for _ in range(n):
                if not fillq:
                    return
                fillq.popleft()()

        # ---------------- work units ----------------
        def unit_ptr(sg, d, pool):
            def run():
                tag = "pj" if pool is ps_pj else "sc"
                ptr = pool.tile([128, 512], F32R, tag=tag, name=f"ptr{sg}_{d}")
                for i in range(4):
                    nc.tensor.transpose(ptr[:, ts(i, 128)], xs_tiles[sg][i][:, ts(d, 128)],
                                        identr)
                nc.vector.tensor_copy(xT[:, d, ds(sg * 512, 512)], ptr)
            return run

        def unit_pkv(sg):
            def run():
                pkv = ps_sc.tile([128, 1024], F32, tag="sc", name=f"pkv{sg}")
                for d in range(DKT):
                    nc.tensor.matmul(pkv[:, 0:512], wkv_sb[:, d, :],
                                     xT[:, d, ds(sg * 512, 512)],
                                     start=(d == 0), stop=(d == DKT - 1))
                nc.vector.tensor_copy(kkT[ds(0, 64), ds(sg * 512, 512)],
                                      pkv[ds(0, 64), 0:512])
                nc.sync.dma_start(out=kkT[ds(64, 64), ds(sg * 512, 512)],
                                  in_=kkT[ds(0, 64), ds(sg * 512, 512)])
                vtmp = vtp.tile([64, 512], F32R, tag="vt", name=f"vt{sg}")
                nc.vector.tensor_copy(vtmp, pkv[ds(64, 64), 0:512])
                vtmp_tiles[sg] = vtmp
            return run

        def unit_ptv(sg):
            def run():
                ptv = ps_pj.tile([128, 512], F32R, tag="pj", name=f"ptv{sg}")
                for i in range(4):
                    nc.tensor.transpose(ptv[:, ds(i * D, D)], vtmp_tiles[sg][:, ts(i, 128)],
                                        identr[0:64, 0:64])
                nc.vector.tensor_copy(vn[:, ds(sg * 4, 4), 0:D], ptv[:, 0:4 * D])
            return run

        def unit_qt(qc, p):
            def run():
                pq = ps_sc.tile([128, 1024], F32, tag="sc", name=f"pq{qc}_{p}")
                for d in range(DKT):
                    nc.tensor.matmul(pq[:, 0:512], wq_sb[:, d, ts(p, 128)],
                                     xT[:, d, ds(qc * 512, 512)],
                                     start=(d == 0), stop=(d == DKT - 1))
                nc.vector.tensor_copy(qt[:, p, ds(qc * 512, 512)], pq[:, 0:512])
            return run

        def unit_po(qc, od):
            def run():
                po = ps_sc.tile([128, 1024], F32, tag="sc", name=f"po{qc}_{od}")
                nc.tensor.matmul(po[:, 0:512], wo_sb[:, 0, ts(od, 128)],
                                 aoutT[:, 0, ds(qc * 512, 512)], start=True, stop=False)
                nc.tensor.matmul(po[:, 0:512], wo_sb[:, 1, ts(od, 128)],
                                 aoutT[:, 1, ds(qc * 512, 512)], start=False, stop=True)
                ot = outp.tile([128, 512], F32, tag="ot", name=f"ot{qc}_{od}")
                nc.vector.tensor_copy(ot, po[:, 0:512])
                nc.sync.dma_start(out=out[ts(od, 128), ds(qc * 512, 512)], in_=ot)
            return run

        # ---------------- attention ----------------
        def flush_pv():
            for (qc_, j_, h_, pt_) in pending_pv:
                hp, hh = h_ // 2, h_ % 2
                for t in range(2):
                    kt = 2 * j_ + t
                    for i in range(4):
                        stn = pt_[:, ds(t * 512 + i * 128, 128)]
                        # start=True zeroes the whole 2KB PSUM bank: only the
                        # first series touching each bank may set it.
                        nc.tensor.matmul(state[qc_][hp][:, hh, i, :], stn,
                                         vn[:, kt, 0:D],
                                         start=(kt == 0 and i == 0 and hh == 0),
                                         stop=(kt == KT - 1),
                                         skip_group_check=True)
                        nc.tensor.matmul(sums[qc_][:, ds(h_ * 4 + i, 1)], stn,
                                         vn[:, kt, D:D + 1],
                                         start=(kt == 0 and i == 0 and h_ == 0),
                                         stop=(kt == KT - 1),
                                         skip_group_check=True)
            pending_pv.clear()

        def emit_quanta(qc, j):
            new_pv = []
            for h in range(4):
                p, i = h // 2, h % 2
                psc = ps_sc.tile([128, 1024], F32, tag="sc", name=f"psc{qc}_{j}_{h}")
                for t in range(2):
                    kt = 2 * j + t
                    nc.tensor.matmul(psc[:, ds(t * 512, 512)],
                                     kkT[ds(i * 64, 64), ts(kt, 128)],
                                     qt[ds(i * 64, 64), p, ds(qc * 512, 512)],
                                     start=True, stop=True)
                pt = ptp.tile([128, 1024], BF16, tag="pt", name=f"pt{qc}_{j}_{h}")
                nc.scalar.activation(out=pt, in_=psc, func=EXPF, scale=SCALE)
                new_pv.append((qc, j, h, pt))
                fill(1)
            flush_pv()
            pending_pv.extend(new_pv)

        def alloc_state(qc):
            state[qc] = [
                ps_pv.tile([128, 2, 4, D], F32, tag="pv", name=f"pv{qc}_{hp}")
                for hp in range(2)
            ]
            sums[qc] = ps_su.tile([128, 16], F32, tag="su", name=f"su{qc}")

        def emit_norm(qc):
            rr = rrp.tile([128, 16], F32, tag="rr", name=f"rr{qc}")
            nc.vector.reciprocal(out=rr, in_=sums[qc])
            ao = aop.tile([128, 4, 4, D], F32R, tag="ao", name=f"ao{qc}")
            for hp in range(2):
                for hh in range(2):
                    h = 2 * hp + hh
                    for i in range(4):
                        nc.vector.tensor_scalar_mul(ao[:, i, h, :],
                                                    state[qc][hp][:, hh, i, :],
                                                    rr[:, ds(h * 4 + i, 1)])
            for hp in range(2):
                pat = ps_pj.tile([128, 512], F32R, tag="pj", name=f"pat{qc}_{hp}")
                for i in range(4):
                    nc.tensor.transpose(pat[:, ds(i * 128, 128)],
                                        ao[:, i, ds(2 * hp, 2), :], identr)
                nc.vector.tensor_copy(aoutT[:, hp, ds(qc * 512, 512)], pat)

        # ---------------- schedule ----------------
        xs_tiles = {}
        vtmp_tiles = {}

        def dma_x(sg):
            xs = [xpool.tile([128, DIM], F32R, tag="xs", name=f"xs{sg}_{i}")
                  for i in range(4)]
            for i in range(4):
                nc.sync.dma_start(out=xs[i], in_=xr[ts(sg * 4 + i, 128), :])
            xs_tiles[sg] = xs

        def sgroup_units(sg, eager):
            dma_x(sg)
            units = []
            for d in range(DKT):
                pool = ps_pj if d % 2 == 0 else ps_sc
                units.append(unit_ptr(sg, d, pool))
            units.append(unit_pkv(sg))
            units.append(unit_ptv(sg))
            if eager:
                for u in units:
                    u()
            else:
                fillq.extend(units)

        # prologue: key chunk 0 eagerly, then qc0 attention interleaved with
        # the remaining key chunks' prep work
        sgroup_units(0, eager=True)
        unit_qt(0, 0)()
        unit_qt(0, 1)()
        alloc_state(0)
        for j in range(KT // 2):
            sg_next = j // 2 + 1
            if j % 2 == 0 and sg_next < QC:
                sgroup_units(sg_next, eager=False)
            if j == 5:
                fillq.append(unit_qt(1, 0))
            if j == 6:
                fillq.append(unit_qt(1, 1))
            emit_quanta(0, j)
            fill(1)
        for qc in range(1, QC):
            emit_quanta(qc, 0)       # flushes (qc-1, 7) into state[qc-1]
            emit_norm(qc - 1)
            alloc_state(qc)
            for od in range(8):
                fillq.append(unit_po(qc - 1, od))
            for j in range(1, KT // 2):
                if j == 5 and qc + 1 < QC:
                    fillq.append(unit_qt(qc + 1, 0))
                if j == 6 and qc + 1 < QC:
                    fillq.append(unit_qt(qc + 1, 1))
                emit_quanta(qc, j)
                fill(1)
        flush_pv()
        emit_norm(QC - 1)
        fill(len(fillq))
        for od in range(8):
            unit_po(QC - 1, od)()

    nc.compile()
    return nc


_CACHE = {}


def _get_nc(NSEQ):
    if NSEQ not in _CACHE:
        _CACHE[NSEQ] = build_nc(NSEQ)
    return _CACHE[NSEQ]


def kernel(x, Wq, Wk, Wv, Wo, bo):
    """Full-input entry point: shard over 8 cores, run, gather."""
    x, Wq, Wk, Wv, Wo, bo = (np.asarray(a, np.float32) for a in (x, Wq, Wk, Wv, Wo, bo))
    B, N, C = x.shape
    nc = _get_nc(N)
    in_maps = []
    for c in range(8):
        b, g = c // 4, c % 4
        in_maps.append({
            "x": np.ascontiguousarray(x[b]),
            "wq": np.ascontiguousarray(Wq[:, g * 256:(g + 1) * 256]),
            "wkv": np.ascontiguousarray(np.concatenate(
                [Wk[:, g * D:(g + 1) * D], Wv[:, g * D:(g + 1) * D]], axis=1)),
            "wo": np.ascontiguousarray(Wo[g * 256:(g + 1) * 256, :]),
        })
    res = bass_utils.run_bass_kernel_spmd(nc, in_maps, core_ids=list(range(8)))
    outs = [res.results[c]["out"] for c in range(8)]
    full = np.empty((B, N, C), np.float32)
    for b in range(B):
        acc = outs[4 * b].astype(np.float32)
        for g in range(1, 4):
            acc = acc + outs[4 * b + g]
        full[b] = acc.T + bo[None, :]
    return full


# revision 11
# speedup vs baseline: 1.2127x; 1.0558x over previous
"""GQA attention kernel for Trainium2 (Bass/Tile), 8-core SPMD.

Problem: B=2, N=2048, DIM=1024, 16 query heads / 4 KV heads, head_dim=64, fp32.
Sharding: core c = (batch b=c//4, kv-group g=c%4). Each core computes its
group's 4 query heads + 1 shared KV head over the full sequence, and a partial
output projection (its 256 rows of Wo). Host sums the 4 group partials per
batch and adds the bias.

Layout per core:
  xT    [128, 8, N] f32r : x^T (PE transposes with an f32r identity)
  qt    [128, 2, N] f32r : Q^T head pairs (head 2p on partitions 0-63, 2p+1 on
                           64-127)
  kkT   [128, N]    f32r : K^T duplicated across partition halves (DMA dup)
  vn    [128, 16, 65] bf16: V in normal layout (keys on partitions) + ones col
  aoutT [128, 2, N] f32r : normalized attention out^T for the out-projection

Scores are computed transposed (S^T [128 keys, 512 queries]); exp on Act; P@V
uses P^T tiles as the *stationary* operand and V as the moving operand,
producing [queries, 64] in PSUM at 64 rows/matmul instead of 128; sum-of-exp
rides on 1-row ones-matmuls into a dedicated PSUM bank.

PSUM budget (8 banks): scores 2x[128,1024] double-buffered (4) + P@V
accumulators 2x[128,512] (2, two heads per bank) + transpose staging (1) +
sum-of-exp (1). Projection matmuls share the score pool, interleaved
fine-grained between score tiles so no engine convoys behind one pool.
"""

import sys

if "/opt/trn_rl_repo" not in sys.path:
    sys.path.insert(0, "/opt/trn_rl_repo")

from collections import deque
from contextlib import ExitStack

import numpy as np

import concourse.bass as bass
import concourse.mybir as mybir
import concourse.tile as tile
from concourse import bacc, bass_utils
from concourse.bass import ds, ts
from concourse.masks import make_identity

F32 = mybir.dt.float32
F32R = mybir.dt.float32r
BF16 = mybir.dt.bfloat16
EXPF = mybir.ActivationFunctionType.Exp

DIM = 1024
D = 64  # head dim
SCALE = D ** -0.5


def build_nc(NSEQ=2048):
    KT = NSEQ // 128   # key tiles
    QC = NSEQ // 512   # query chunks of 512
    DKT = DIM // 128   # contraction tiles for projections

    nc = bacc.Bacc("TRN2", target_bir_lowering=False, debug=False)
    x = nc.dram_tensor("x", [NSEQ, DIM], F32, kind="ExternalInput").ap()
    wq = nc.dram_tensor("wq", [DIM, 256], F32, kind="ExternalInput").ap()
    wkv = nc.dram_tensor("wkv", [DIM, 128], F32, kind="ExternalInput").ap()
    wo = nc.dram_tensor("wo", [256, DIM], F32, kind="ExternalInput").ap()
    out = nc.dram_tensor("out", [DIM, NSEQ], F32, kind="ExternalOutput").ap()

    xr = x.bitcast(F32R)

    with tile.TileContext(nc) as tc, ExitStack() as ctx:
        sb = ctx.enter_context(tc.tile_pool(name="sb", bufs=1))

        wq_sb = sb.tile([128, DKT, 256], F32R)
        wkv_sb = sb.tile([128, DKT, 128], F32R)
        wo_sb = sb.tile([128, 2, DIM], F32R)
        ident = sb.tile([128, 128], F32)
        identr = sb.tile([128, 128], F32R)
        warm_in = sb.tile([128, 1], F32)
        warm = sb.tile([128, 1], F32)

        make_identity(nc, ident)
        nc.vector.tensor_copy(identr, ident)
        nc.vector.memset(warm_in, 1.0)
        # preload the exp table set off the critical path
        nc.scalar.activation(out=warm, in_=warm_in, func=EXPF, scale=1.0)

        xT = sb.tile([128, DKT, NSEQ], F32R)
        qt = sb.tile([128, 2, NSEQ], F32R)
        kkT = sb.tile([128, NSEQ], F32R)
        vn = sb.tile([128, KT, D + 1], BF16)
        aoutT = sb.tile([128, 2, NSEQ], F32R)
        nc.vector.memset(vn, 1.0)

        xpool = ctx.enter_context(tc.tile_pool(name="xp", bufs=6))
        vtp = ctx.enter_context(tc.tile_pool(name="vtp", bufs=2))
        ptp = ctx.enter_context(tc.tile_pool(name="ptp", bufs=10))
        rrp = ctx.enter_context(tc.tile_pool(name="rrp", bufs=2))
        aop = ctx.enter_context(tc.tile_pool(name="aop", bufs=2))
        outp = ctx.enter_context(tc.tile_pool(name="outp", bufs=3))
        # PSUM: ps_sc 2x[128,1024] (banks 0-3), ps_pv 2x[128,512] (4-5),
        # ps_pj 1x[128,512] (6), ps_su 1x[128,16] (7)
        ps_sc = ctx.enter_context(tc.tile_pool(name="ps_sc", bufs=2, space="PSUM"))
        ps_pv = ctx.enter_context(tc.tile_pool(name="ps_pv", bufs=2, space="PSUM"))
        ps_pj = ctx.enter_context(tc.tile_pool(name="ps_pj", bufs=1, space="PSUM"))
        ps_su = ctx.enter_context(tc.tile_pool(name="ps_su", bufs=1, space="PSUM"))

        state = {}     # qc -> [hp0_tile, hp1_tile] each [128, 2, 4, 64] view
        sums = {}      # qc -> [128, 16] psum tile (cols h*4+i)
        pending_pv = []
        fillq = deque()

        def fill(n=1):
            for _ in range(n):
                if not fillq:
                    return
                fillq.popleft()()

        # ---------------- work units ----------------
        def unit_ptr2(sg, dp):
            """Transpose two DIM-tiles of x chunk sg into one 2-bank psum."""
            def run():
                ptr = ps_sc.tile([128, 1024], F32R, tag="sc", name=f"ptr{sg}_{dp}")
                for k in range(2):
                    d = 2 * dp + k
                    for i in range(4):
                        nc.tensor.transpose(ptr[:, ds(k * 512 + i * 128, 128)],
                                            xs_tiles[sg][i][:, ts(d, 128)], identr)
                nc.vector.tensor_copy(xT[:, ds(2 * dp, 2), ds(sg * 512, 512)], ptr)
            return run

        def unit_ptr1(sg, d):
            def run():
                ptr = ps_pj.tile([128, 512], F32R, tag="pj", name=f"ptr{sg}_{d}")
                for i in range(4):
                    nc.tensor.transpose(ptr[:, ts(i, 128)], xs_tiles[sg][i][:, ts(d, 128)],
                                        identr)
                nc.vector.tensor_copy(xT[:, d, ds(sg * 512, 512)], ptr)
            return run

        def unit_pkv(sg):
            def run():
                pkv = ps_sc.tile([128, 1024], F32, tag="sc", name=f"pkv{sg}")
                for d in range(DKT):
                    nc.tensor.matmul(pkv[:, 0:512], wkv_sb[:, d, :],
                                     xT[:, d, ds(sg * 512, 512)],
                                     start=(d == 0), stop=(d == DKT - 1))
                nc.vector.tensor_copy(kkT[ds(0, 64), ds(sg * 512, 512)],
                                      pkv[ds(0, 64), 0:512])
                nc.sync.dma_start(out=kkT[ds(64, 64), ds(sg * 512, 512)],
                                  in_=kkT[ds(0, 64), ds(sg * 512, 512)])
                vtmp = vtp.tile([64, 512], F32R, tag="vt", name=f"vt{sg}")
                nc.vector.tensor_copy(vtmp, pkv[ds(64, 64), 0:512])
                vtmp_tiles[sg] = vtmp
            return run

        def unit_ptv(sg):
            def run():
                ptv = ps_pj.tile([128, 512], F32R, tag="pj", name=f"ptv{sg}")
                for i in range(4):
                    nc.tensor.transpose(ptv[:, ds(i * D, D)], vtmp_tiles[sg][:, ts(i, 128)],
                                        identr[0:64, 0:64])
                nc.vector.tensor_copy(vn[:, ds(sg * 4, 4), 0:D], ptv[:, 0:4 * D])
            return run

        def unit_qt(qc, p):
            def run():
                pq = ps_sc.tile([128, 1024], F32, tag="sc", name=f"pq{qc}_{p}")
                for d in range(DKT):
                    nc.tensor.matmul(pq[:, 0:512], wq_sb[:, d, ts(p, 128)],
                                     xT[:, d, ds(qc * 512, 512)],
                                     start=(d == 0), stop=(d == DKT - 1))
                nc.vector.tensor_copy(qt[:, p, ds(qc * 512, 512)], pq[:, 0:512])
            return run

        def unit_po(qc, od, pool=None):
            def run():
                p_ = pool if pool is not None else ps_sc
                tag = "pj" if p_ is ps_pj else "sc"
                shape = [128, 512] if p_ is ps_pj else [128, 1024]
                po = p_.tile(shape, F32, tag=tag, name=f"po{qc}_{od}")
                nc.tensor.matmul(po[:, 0:512], wo_sb[:, 0, ts(od, 128)],
                                 aoutT[:, 0, ds(qc * 512, 512)], start=True, stop=False)
                nc.tensor.matmul(po[:, 0:512], wo_sb[:, 1, ts(od, 128)],
                                 aoutT[:, 1, ds(qc * 512, 512)], start=False, stop=True)
                ot = outp.tile([128, 512], F32, tag="ot", name=f"ot{qc}_{od}")
                nc.vector.tensor_copy(ot, po[:, 0:512])
                nc.sync.dma_start(out=out[ts(od, 128), ds(qc * 512, 512)], in_=ot)
            return run

        # ---------------- attention ----------------
        def flush_pv():
            for (qc_, j_, h_, pt_) in pending_pv:
                hp, hh = h_ // 2, h_ % 2
                for t in range(2):
                    kt = 2 * j_ + t
                    for i in range(4):
                        stn = pt_[:, ds(t * 512 + i * 128, 128)]
                        # start=True zeroes the whole 2KB PSUM bank: only the
                        # first series touching each bank may set it.
                        nc.tensor.matmul(state[qc_][hp][:, hh, i, :], stn,
                                         vn[:, kt, 0:D],
                                         start=(kt == 0 and i == 0 and hh == 0),
                                         stop=(kt == KT - 1),
                                         skip_group_check=True)
                        nc.tensor.matmul(sums[qc_][:, ds(h_ * 4 + i, 1)], stn,
                                         vn[:, kt, D:D + 1],
                                         start=(kt == 0 and i == 0 and h_ == 0),
                                         stop=(kt == KT - 1),
                                         skip_group_check=True)
            pending_pv.clear()

        def emit_quanta(qc, j, mid_fills=(1, 3)):
            new_pv = []
            for h in range(4):
                p, i = h // 2, h % 2
                psc = ps_sc.tile([128, 1024], F32, tag="sc", name=f"psc{qc}_{j}_{h}")
                for t in range(2):
                    kt = 2 * j + t
                    nc.tensor.matmul(psc[:, ds(t * 512, 512)],
                                     kkT[ds(i * 64, 64), ts(kt, 128)],
                                     qt[ds(i * 64, 64), p, ds(qc * 512, 512)],
                                     start=True, stop=True)
                pt = ptp.tile([128, 1024], BF16, tag="pt", name=f"pt{qc}_{j}_{h}")
                nc.scalar.activation(out=pt, in_=psc, func=EXPF, scale=SCALE)
                new_pv.append((qc, j, h, pt))
                if h in mid_fills:
                    fill(1)
            flush_pv()
            pending_pv.extend(new_pv)

        def alloc_state(qc):
            state[qc] = [
                ps_pv.tile([128, 2, 4, D], F32, tag="pv", name=f"pv{qc}_{hp}")
                for hp in range(2)
            ]
            sums[qc] = ps_su.tile([128, 16], F32, tag="su", name=f"su{qc}")

        def emit_norm(qc):
            rr = rrp.tile([128, 16], F32, tag="rr", name=f"rr{qc}")
            nc.vector.reciprocal(out=rr, in_=sums[qc])
            ao = aop.tile([128, 4, 4, D], F32R, tag="ao", name=f"ao{qc}")
            for hp in range(2):
                for hh in range(2):
                    h = 2 * hp + hh
                    for i in range(4):
                        nc.vector.tensor_scalar_mul(ao[:, i, h, :],
                                                    state[qc][hp][:, hh, i, :],
                                                    rr[:, ds(h * 4 + i, 1)])
            for hp in range(2):
                pat = ps_pj.tile([128, 512], F32R, tag="pj", name=f"pat{qc}_{hp}")
                for i in range(4):
                    nc.tensor.transpose(pat[:, ds(i * 128, 128)],
                                        ao[:, i, ds(2 * hp, 2), :], identr)
                nc.vector.tensor_copy(aoutT[:, hp, ds(qc * 512, 512)], pat)

        # ---------------- schedule ----------------
        xs_tiles = {}
        vtmp_tiles = {}

        def dma_x(sg):
            if sg in xs_tiles:
                return
            xs = [xpool.tile([128, DIM], F32R, tag="xs", name=f"xs{sg}_{i}")
                  for i in range(4)]
            for i in range(4):
                nc.sync.dma_start(out=xs[i], in_=xr[ts(sg * 4 + i, 128), :])
            xs_tiles[sg] = xs

        def sgroup_units(sg, eager):
            dma_x(sg)
            units = [unit_ptr2(sg, 0), unit_ptr1(sg, 4), unit_ptr2(sg, 1),
                     unit_ptr1(sg, 5), unit_ptr1(sg, 6), unit_ptr1(sg, 7),
                     unit_pkv(sg), unit_ptv(sg)]
            if eager:
                for u in units:
                    u()
            else:
                fillq.extend(units)

        # prologue: x chunk 0 first on the DMA engines, then weights
        dma_x(0)
        nc.sync.dma_start(out=wkv_sb, in_=wkv.rearrange("(t p) m -> p t m", p=128).bitcast(F32R))
        nc.sync.dma_start(out=wq_sb, in_=wq.rearrange("(t p) m -> p t m", p=128).bitcast(F32R))
        sgroup_units(0, eager=True)
        unit_qt(0, 0)()
        unit_qt(0, 1)()
        nc.sync.dma_start(out=wo_sb, in_=wo.rearrange("(t p) m -> p t m", p=128).bitcast(F32R))
        alloc_state(0)
        for j in range(KT // 2):
            sg_next = j // 2 + 1
            if j % 2 == 0 and sg_next < QC:
                sgroup_units(sg_next, eager=False)
            if j == 5:
                fillq.append(unit_qt(1, 0))
            if j == 6:
                fillq.append(unit_qt(1, 1))
            emit_quanta(0, j, mid_fills=(0, 1, 2, 3))
            fill(1)
        for qc in range(1, QC):
            emit_quanta(qc, 0)       # flushes (qc-1, 7) into state[qc-1]
            emit_norm(qc - 1)
            alloc_state(qc)
            for j in range(1, KT // 2):
                fillq.append(unit_po(qc - 1, j - 1))
                if j == 5 and qc + 1 < QC:
                    fillq.append(unit_qt(qc + 1, 0))
                if j == 6 and qc + 1 < QC:
                    fillq.append(unit_qt(qc + 1, 1))
                if j == 7:
                    fillq.append(unit_po(qc - 1, 7))
                emit_quanta(qc, j)
                fill(1)
        flush_pv()
        emit_norm(QC - 1)
        fill(len(fillq))
        for od in range(8):
            pool = ps_pj if od % 3 == 2 else ps_sc
            unit_po(QC - 1, od, pool)()

    nc.compile()
    return nc


_CACHE = {}


def _get_nc(NSEQ):
    if NSEQ not in _CACHE:
        _CACHE[NSEQ] = build_nc(NSEQ)
    return _CACHE[NSEQ]


def kernel(x, Wq, Wk, Wv, Wo, bo):
    """Full-input entry point: shard over 8 cores, run, gather."""
    x, Wq, Wk, Wv, Wo, bo = (np.asarray(a, np.float32) for a in (x, Wq, Wk, Wv, Wo, bo))
    B, N, C = x.shape
    nc = _get_nc(N)
    in_maps = []
    for c in range(8):
        b, g = c // 4, c % 4
        in_maps.append({
            "x": np.ascontiguousarray(x[b]),
            "wq": np.ascontiguousarray(Wq[:, g * 256:(g + 1) * 256]),
            "wkv": np.ascontiguousarray(np.concatenate(
                [Wk[:, g * D:(g + 1) * D], Wv[:, g * D:(g + 1) * D]], axis=1)),
            "wo": np.ascontiguousarray(Wo[g * 256:(g + 1) * 256, :]),
        })
    res = bass_utils.run_bass_kernel_spmd(nc, in_maps, core_ids=list(range(8)))
    outs = [res.results[c]["out"] for c in range(8)]
    full = np.empty((B, N, C), np.float32)
    for b in range(B):
        acc = outs[4 * b].astype(np.float32)
        for g in range(1, 4):
            acc = acc + outs[4 * b + g]
        full[b] = acc.T + bo[None, :]
    return full
